# revision 1
# baseline (speedup 1.0000x reference)
"""KAN B-spline activation kernel for Trainium2 (8 NeuronCores, data-parallel on batch).

Math (validated vs reference to ~1e-7 rel):
  grid is uniform: g[t] = -1 + (t-3)*h, h = 0.125, t = 0..22; u = (x - g[0])/h = 8x + 11.
  For x in [0,1) only knot-window t in [8,18] has nonzero cubic bases.
  Let A[k] = x - g[8+k], k = 0..15 (k=15 unused pad).
  B1[m]  = Relu(1 - |A[m+1]|/h)                     (hat; == Cox-de Boor level 1), m=0..12
  B2d[m] = A[m]*B1[m]   - A[m+3]*B1[m+1]           (= 2h * B2), m=0..11
  B3d[m] = A[m]*B2d[m]  - A[m+4]*B2d[m+1]          (= 6h^2 * B3), m=0..10
  out[b,o,i] = sum_m B3d[b,i,m] * coef[o,i,8+m] / (6 h^2)

Device layout (per core, 128 batch rows in partitions):
  A/B* tiles: (128, 64 inputs x 16-knot-window blocks) in the free dim.
  B3 (128, 64*16) -> 8 PE transposes of 128-col groups (8 inputs each) ->
  basesT (K=(input,knot) partitions, batch free). Matmul per (group g, subgroup s):
  K=32 (2 inputs x 16 knots), N=128 (2 inputs x 64 outs), block-diagonal rhs built
  on host with the zeros/padding baked in. PSUM bank per group (128,512) is copied
  verbatim to SBUF and DMA'd out; host un-permutes (b, g, s, p, o) -> (b, o, i).
"""

import numpy as np
from contextlib import ExitStack

import concourse.bass as bass
import concourse.tile as tile
from concourse import bacc, mybir
from concourse.bass_utils import run_bass_kernel_spmd
from concourse.masks import make_identity

N_CORES = 8
B_TOT, IN_DIM, OUT_DIM = 1024, 64, 64
BPC = B_TOT // N_CORES          # 128 batch rows per core
K16 = 16                        # padded knot window per input
NG = 8                          # groups of 8 inputs
F32 = mybir.dt.float32

# If the stride-0 broadcast read on DVE fails, flip to False (log-doubling copies).
# HW faulted with stride-0 input APs on DVE (sim accepts them); use doubling.
USE_STRIDE0 = False

_CACHE = {}


def _build_nc():
    nc = bacc.Bacc("TRN2", target_bir_lowering=False, debug=False,
                   num_devices=N_CORES)
    x_d = nc.dram_tensor("x_in", [BPC, IN_DIM], F32, kind="ExternalInput").ap()
    rhs_d = nc.dram_tensor("rhs_in", [128, NG * 512], F32, kind="ExternalInput").ap()
    g3_d = nc.dram_tensor("g3_in", [1, IN_DIM * K16], F32, kind="ExternalInput").ap()
    out_d = nc.dram_tensor("out", [BPC, NG, 512], F32, kind="ExternalOutput").ap()

    with tile.TileContext(nc) as tc, ExitStack() as ctx:
        pool = ctx.enter_context(tc.tile_pool(name="main", bufs=1))
        psT = ctx.enter_context(tc.tile_pool(name="psT", bufs=2, space="PSUM"))
        psO = ctx.enter_context(tc.tile_pool(name="psO", bufs=4, space="PSUM"))
        og_pool = ctx.enter_context(tc.tile_pool(name="og", bufs=4))

        ident = pool.tile([128, 128], F32)
        make_identity(nc, ident)

        x_sb = pool.tile([BPC, IN_DIM], F32)
        nc.sync.dma_start(out=x_sb[:], in_=x_d)
        rhs_sb = pool.tile([128, NG * 512], F32)
        nc.sync.dma_start(out=rhs_sb[:], in_=rhs_d)
        # broadcast the (1, 1024) knot row across 128 partitions during DMA
        g3_sb = pool.tile([128, IN_DIM * K16], F32)
        g3_bcast = bass.AP(tensor=g3_d.tensor, offset=g3_d.offset,
                           ap=[[0, 128]] + list(g3_d.ap[1:]))
        nc.gpsimd.dma_start(out=g3_sb[:], in_=g3_bcast)
        g3v = g3_sb[:].rearrange("p (i k) -> p i k", k=K16)

        # broadcast x along the 16-knot window by log-doubling copies
        xt = pool.tile([BPC, IN_DIM, K16], F32)
        nc.vector.tensor_copy(xt[:, :, 0:1],
                              x_sb[:].rearrange("p (i k) -> p i k", k=1))
        w = 1
        while w < K16:
            n = min(w, K16 - w)
            nc.vector.tensor_copy(xt[:, :, w:w + n], xt[:, :, 0:n])
            w += n

        halves = ctx.enter_context(tc.tile_pool(name="halves", bufs=2))
        basesT = pool.tile([128, NG * 128], F32)
        HW_IN = IN_DIM // 2                       # 32 inputs per half
        for H in range(2):
            isl = slice(H * HW_IN, (H + 1) * HW_IN)
            Ah = halves.tile([BPC, HW_IN, K16], F32)
            nc.vector.tensor_sub(Ah[:], xt[:, isl, :], g3v[:, isl, :])
            Bab = halves.tile([BPC, HW_IN, 13], F32)
            nc.scalar.activation(out=Bab[:], in_=Ah[:, :, 1:14],
                                 func=mybir.ActivationFunctionType.Abs)
            B1h = halves.tile([BPC, HW_IN, 13], F32)
            # Relu(-8*|A| + 1) == Relu(1 - |A|/h)
            nc.scalar.activation(out=B1h[:], in_=Bab[:],
                                 func=mybir.ActivationFunctionType.Relu,
                                 scale=-8.0, bias=1.0)
            Ml2 = halves.tile([BPC, HW_IN, 12], F32)
            Mr2 = halves.tile([BPC, HW_IN, 12], F32)
            B2h = halves.tile([BPC, HW_IN, 12], F32)
            nc.vector.tensor_mul(Ml2[:], Ah[:, :, 0:12], B1h[:, :, 0:12])
            nc.vector.tensor_mul(Mr2[:], Ah[:, :, 3:15], B1h[:, :, 1:13])
            nc.vector.tensor_sub(B2h[:], Ml2[:], Mr2[:])
            Ml3 = halves.tile([BPC, HW_IN, 11], F32)
            Mr3 = halves.tile([BPC, HW_IN, 11], F32)
            B3h = halves.tile([BPC, HW_IN, K16], F32)
            nc.vector.tensor_mul(Ml3[:], Ah[:, :, 0:11], B2h[:, :, 0:11])
            nc.vector.tensor_mul(Mr3[:], Ah[:, :, 4:15], B2h[:, :, 1:12])
            # pad knots 11..15 must be 0: they feed the transpose, whose
            # output multiplies real coef columns.
            nc.vector.memset(B3h[:, :, 11:16], 0.0)
            nc.vector.tensor_sub(B3h[:, :, 0:11], Ml3[:], Mr3[:])

            B3f = B3h[:].rearrange("p i k -> p (i k)")
            ps_t = psT.tile([128, 512], F32)
            for q in range(4):
                nc.tensor.transpose(out=ps_t[:, q * 128:(q + 1) * 128],
                                    in_=B3f[:, q * 128:(q + 1) * 128],
                                    identity=ident[:])
            dst = basesT[:, H * 512:(H + 1) * 512]
            if H == 0:
                nc.vector.tensor_copy(dst, ps_t[:])
            else:
                nc.scalar.copy(dst, ps_t[:])

            for q in range(4):
                g = 4 * H + q
                ps_o = psO.tile([128, 512], F32)
                nc.tensor.matmul(out=ps_o[:],
                                 lhsT=basesT[:, g * 128:(g + 1) * 128],
                                 rhs=rhs_sb[:, g * 512:(g + 1) * 512],
                                 start=True, stop=True)
                og = og_pool.tile([128, 512], F32)
                if g % 2 == 0:
                    nc.vector.tensor_copy(og[:], ps_o[:])
                else:
                    nc.scalar.copy(og[:], ps_o[:])
                nc.sync.dma_start(out=out_d[:, g, :], in_=og[:])

    nc.compile()
    return nc


def _host_inputs(x, coef, grid):
    x = np.ascontiguousarray(np.asarray(x, dtype=np.float32))
    coef = np.asarray(coef, dtype=np.float32)
    knots = np.asarray(grid, dtype=np.float32)[0, 0, :]          # (23,)
    h = float(knots[1] - knots[0])

    g3 = np.empty(K16, dtype=np.float32)
    g3[:15] = knots[8:23]
    g3[15] = knots[22] + h                                       # unused pad
    g3row = np.tile(g3, IN_DIM)[None, :]                         # (1, 1024)

    scale = 1.0 / (6.0 * h * h)
    cf = coef[:, :, 8:19] * scale                                # (o, i, 11)
    # block-diagonal rhs per group: rows (i_l,j) x cols (i_l', o), K=128, N=512
    rhs = np.zeros((128, NG * 512), dtype=np.float32)
    for i_l in range(8):
        for g in range(NG):
            i = g * 8 + i_l
            rhs[i_l * 16:i_l * 16 + 11,
                g * 512 + i_l * 64:g * 512 + i_l * 64 + 64] = cf[:, i, :].T
    return x, rhs, g3row


def _execute(x, coef, grid, trace=False, **spmd_kwargs):
    xf, rhs, g3row = _host_inputs(x, coef, grid)
    if "nc" not in _CACHE:
        _CACHE["nc"] = _build_nc()
    nc = _CACHE["nc"]
    in_maps = [{"x_in": np.ascontiguousarray(xf[c * BPC:(c + 1) * BPC]),
                "rhs_in": rhs, "g3_in": g3row} for c in range(N_CORES)]
    res = run_bass_kernel_spmd(nc, in_maps, list(range(N_CORES)),
                               trace=trace, **spmd_kwargs)
    full = np.empty((B_TOT, OUT_DIM, IN_DIM), dtype=np.float32)
    for c in range(N_CORES):
        t = res.results[c]["out"].reshape(BPC, NG, 8, 64)        # (b, g, i_l, o)
        full[c * BPC:(c + 1) * BPC] = (
            t.transpose(0, 3, 1, 2).reshape(BPC, OUT_DIM, IN_DIM))
    return full, res


def kernel(x, coef, grid):
    out, _ = _execute(x, coef, grid, trace=False)
    return out



# revision 4
# speedup vs baseline: 1.4025x; 1.4025x over previous
"""KAN B-spline activation kernel for Trainium2 (8 NeuronCores, data-parallel on batch).

Math (validated vs reference: ~1e-7 in fp64, ~1.1e-3 with the full fp16 chain):
  grid is uniform: g3[k] = knots[8+k], h = spacing; for x in [0,1) only the
  11 cubic bases b3[8..18] are nonzero.
  A[k]     = x - g3[k],                                          k = 0..14
  B1raw[m] = min(A[m], -A[m+2])            (= h * hat_m, pre-relu), m = 0..12
  B2raw[m] = A[m]*relu(B1raw[m]) - A[m+3]*relu(B1raw[m+1])         m = 0..11
  B3raw[m] = A[m]*B2raw[m] - A[m+4]*B2raw[m+1]   (= 6h^3*b3[m+8]),  m = 0..10
  out[b,o,i] = sum_m B3raw[b,i,m] * coef[o,i,8+m] / (6 h^3)

Device schedule (per core, 128 batch rows in partitions):
  - no grid tensor: g3 constants are baked into tensor_scalar immediates.
  - whole elementwise chain in fp16 on DVE as TensorScalarPtr ops
    (scalar_tensor_tensor), which hit the 4x_2p DVE fast mode; the relu is
    folded into the B2 products via (B1raw max 0) mult A.
  - 8 PE transposes of 128-col groups -> basesT ((input,knot) partitions,
    batch free) in fp16; PSUM->SBUF bases copies split scalar/vector.
  - 8 fp16 matmuls: K=128 (8 inputs x 16 knots), N=512 (8 inputs x 64 outs),
    block-diagonal fp16 rhs built on host (zeros baked in).
  - PSUM (fp32) -> og (fp16) copies alternate scalar/vector; 2 output DMAs.
    Host upcasts fp16 -> fp32 and un-permutes (b, g, i_l, o) -> (b, o, i).
"""

import numpy as np
from contextlib import ExitStack

import concourse.bass as bass
import concourse.tile as tile
from concourse import bacc, mybir
from concourse.bass_utils import run_bass_kernel_spmd
from concourse.masks import make_identity

N_CORES = 8
B_TOT, IN_DIM, OUT_DIM = 1024, 64, 64
BPC = B_TOT // N_CORES          # 128 batch rows per core
K16 = 16                        # padded knot window per input
NG = 8                          # groups of 8 inputs
F32 = mybir.dt.float32
F16 = mybir.dt.float16

_CACHE = {}


def _build_nc(g3_0, h):
    AL = mybir.AluOpType
    nc = bacc.Bacc("TRN2", target_bir_lowering=False, debug=False,
                   num_devices=N_CORES)
    x_d = nc.dram_tensor("x_in", [BPC, IN_DIM], F16, kind="ExternalInput").ap()
    rhs_d = nc.dram_tensor("rhs_in", [128, NG * 512], F16,
                           kind="ExternalInput").ap()
    out_d = nc.dram_tensor("out", [BPC, NG * 512], F16,
                           kind="ExternalOutput").ap()

    with tile.TileContext(nc) as tc, ExitStack() as ctx:
        pool = ctx.enter_context(tc.tile_pool(name="main", bufs=1))
        psT = ctx.enter_context(tc.tile_pool(name="psT", bufs=1, space="PSUM"))
        psO = ctx.enter_context(tc.tile_pool(name="psO", bufs=4, space="PSUM"))

        ident = pool.tile([128, 128], F16)
        make_identity(nc, ident)

        x_sb = pool.tile([BPC, IN_DIM], F16)
        nc.sync.dma_start(out=x_sb[:], in_=x_d)
        rhs_sb = pool.tile([128, NG * 512], F16)
        # issue from the scalar queue so SP's x DMA is not delayed
        nc.scalar.dma_start(out=rhs_sb[:, :4 * 512], in_=rhs_d[:, :4 * 512])
        nc.scalar.dma_start(out=rhs_sb[:, 4 * 512:], in_=rhs_d[:, 4 * 512:])

        A = pool.tile([BPC, IN_DIM, 15], F16)
        B1 = pool.tile([BPC, IN_DIM, 13], F16)
        Ml2 = pool.tile([BPC, IN_DIM, 12], F16)
        Mr2 = pool.tile([BPC, IN_DIM, 12], F16)
        B2 = pool.tile([BPC, IN_DIM, 12], F16)
        Ml3 = pool.tile([BPC, IN_DIM, 11], F16)
        Mr3 = pool.tile([BPC, IN_DIM, 11], F16)
        B3h = pool.tile([BPC, IN_DIM, K16], F16)

        # pad knots 11..15 feed the transpose -> real coef columns: keep 0
        nc.gpsimd.memset(B3h[:, :, 11:16], 0.0)

        V = nc.vector
        xv = x_sb[:].rearrange("p (i k) -> p i k", k=1)
        # A[k] = x - g3[k]; k=0..2 from x, the rest as shifted copies
        for k in range(3):
            V.tensor_scalar_sub(A[:, :, k:k + 1], xv, g3_0 + k * h)
        V.tensor_scalar_sub(A[:, :, 3:6], A[:, :, 0:3], 3.0 * h)
        V.tensor_scalar_sub(A[:, :, 6:12], A[:, :, 0:6], 6.0 * h)
        V.tensor_scalar_sub(A[:, :, 12:15], A[:, :, 0:3], 12.0 * h)

        # B1raw[m] = min(-A[m+2], A[m])
        V.scalar_tensor_tensor(B1[:], A[:, :, 2:15], -1.0, A[:, :, 0:13],
                               AL.mult, AL.min)
        # Ml2 = relu(B1raw[m]) * A[m];  Mr2 = relu(B1raw[m+1]) * A[m+3]
        V.scalar_tensor_tensor(Ml2[:], B1[:, :, 0:12], 0.0, A[:, :, 0:12],
                               AL.max, AL.mult)
        V.scalar_tensor_tensor(Mr2[:], B1[:, :, 1:13], 0.0, A[:, :, 3:15],
                               AL.max, AL.mult)
        V.scalar_tensor_tensor(B2[:], Mr2[:], -1.0, Ml2[:], AL.mult, AL.add)
        V.scalar_tensor_tensor(Ml3[:], A[:, :, 0:11], 0.0, B2[:, :, 0:11],
                               AL.add, AL.mult)
        V.scalar_tensor_tensor(Mr3[:], A[:, :, 4:15], 0.0, B2[:, :, 1:12],
                               AL.add, AL.mult)
        V.scalar_tensor_tensor(B3h[:, :, 0:11], Mr3[:], -1.0, Ml3[:],
                               AL.mult, AL.add)

        basesT = pool.tile([128, NG * 128], F16)
        og = pool.tile([BPC, NG * 512], F16)
        ps0 = psT.tile([128, 4 * 128], F16)
        ps1 = psT.tile([128, 4 * 128], F16)

        B3f = B3h[:].rearrange("p i k -> p (i k)")
        for q in range(4):
            nc.tensor.transpose(out=ps0[:, q * 128:(q + 1) * 128],
                                in_=B3f[:, q * 128:(q + 1) * 128],
                                identity=ident[:])
        for q in range(4):
            nc.tensor.transpose(out=ps1[:, q * 128:(q + 1) * 128],
                                in_=B3f[:, (4 + q) * 128:(5 + q) * 128],
                                identity=ident[:])

        nc.vector.tensor_copy(basesT[:, :4 * 128], ps0[:])
        nc.scalar.copy(basesT[:, 4 * 128:], ps1[:])

        cp = [nc.vector.tensor_copy, nc.scalar.copy]
        for g in range(NG):
            ps_o = psO.tile([128, 512], F32)
            nc.tensor.matmul(out=ps_o[:],
                             lhsT=basesT[:, g * 128:(g + 1) * 128],
                             rhs=rhs_sb[:, g * 512:(g + 1) * 512],
                             start=True, stop=True)
            cp[g % 2](og[:, g * 512:(g + 1) * 512], ps_o[:])

        nc.sync.dma_start(out=out_d[:, :4 * 512], in_=og[:, :4 * 512])
        nc.sync.dma_start(out=out_d[:, 4 * 512:], in_=og[:, 4 * 512:])

    nc.compile()
    return nc


def _host_inputs(x, coef, grid):
    x = np.asarray(x, dtype=np.float32).astype(np.float16)
    coef = np.asarray(coef, dtype=np.float32)
    knots = np.asarray(grid, dtype=np.float32)[0, 0, :]          # (23,)
    h = float(knots[1] - knots[0])
    g3_0 = float(knots[8])

    scale = 1.0 / (6.0 * h * h * h)
    cf = coef[:, :, 8:19].astype(np.float64) * scale             # (o, i, 11)
    # block-diagonal rhs per group: rows (i_l,m) x cols (i_l', o), K=128, N=512
    rhs = np.zeros((128, NG * 512), dtype=np.float16)
    for i_l in range(8):
        for g in range(NG):
            i = g * 8 + i_l
            rhs[i_l * 16:i_l * 16 + 11,
                g * 512 + i_l * 64:g * 512 + i_l * 64 + 64] = (
                    cf[:, i, :].T.astype(np.float16))
    return x, rhs, g3_0, h


def _execute(x, coef, grid, trace=False, **spmd_kwargs):
    xf, rhs, g3_0, h = _host_inputs(x, coef, grid)
    if "nc" not in _CACHE:
        _CACHE["nc"] = _build_nc(g3_0, h)
    nc = _CACHE["nc"]
    in_maps = [{"x_in": np.ascontiguousarray(xf[c * BPC:(c + 1) * BPC]),
                "rhs_in": rhs} for c in range(N_CORES)]
    res = run_bass_kernel_spmd(nc, in_maps, list(range(N_CORES)),
                               trace=trace, **spmd_kwargs)
    full = np.empty((B_TOT, OUT_DIM, IN_DIM), dtype=np.float32)
    for c in range(N_CORES):
        t = res.results[c]["out"].astype(np.float32)
        t = t.reshape(BPC, NG, 8, 64)                            # (b, g, i_l, o)
        full[c * BPC:(c + 1) * BPC] = (
            t.transpose(0, 3, 1, 2).reshape(BPC, OUT_DIM, IN_DIM))
    return full, res


def kernel(x, coef, grid):
    out, _ = _execute(x, coef, grid, trace=False)
    return out


# revision 7
# speedup vs baseline: 1.5725x; 1.1212x over previous
"""KAN B-spline activation kernel for Trainium2 (8 NeuronCores, data-parallel on batch).

Math: for the uniform grid (spacing h, g3[k] = knots[8+k]) and x in [0,1),
only cubic bases b3[8..18] are nonzero, and each is the cardinal B-spline
kernel K evaluated at the distance to its center:
  sigma[m] = |x - g3[m+2]| / h                                  m = 0..11
  6*b3[m+8] = relu(2-sigma)^3 - 4*relu(1-sigma)^3  (= T[m]; T[11] == 0)
  out[b,o,i] = sum_m T[b,i,m] * coef[o,i,8+m] / 6
Validated vs reference: 1.5e-3 rel err with the full fp16 chain.

Device schedule (per core, 128 batch rows in partitions):
  - no grid tensor: constants are baked into tensor_scalar immediates.
  - chain in fp16 on DVE; tensor_scalar ops (single-src) hit the 4x DVE
    mode, tensor_tensor (the 4 cube products + final combine) hit 2x; all
    tiles are 12-wide so every row is 4B-aligned with even counts.
  - relu legs as min(sigma-c, 0) = -relu(c-sigma); signs cancel in cubes:
    T = 4*cu1 - cu2.
  - cubes/combine split per half so PE transposes start early.
  - 8 PE transposes of 128-col groups -> basesT ((input,knot) partitions,
    batch free) fp16; 8 fp16 matmuls (K=128, N=512) with a block-diagonal
    fp16 rhs built on host; PSUM->SBUF copies alternate scalar/vector;
    4 rolling output DMAs. Host upcasts fp16 and un-permutes.
"""

import numpy as np
from contextlib import ExitStack

import concourse.bass as bass
import concourse.tile as tile
from concourse import bacc, mybir
from concourse.bass_utils import run_bass_kernel_spmd
from concourse.masks import make_identity

N_CORES = 8
B_TOT, IN_DIM, OUT_DIM = 1024, 64, 64
BPC = B_TOT // N_CORES          # 128 batch rows per core
K16 = 16                        # padded knot window per input
NG = 8                          # groups of 8 inputs
F32 = mybir.dt.float32
F16 = mybir.dt.float16

_CACHE = {}


def _build_nc(g3_2, h):
    AL = mybir.AluOpType
    nc = bacc.Bacc("TRN2", target_bir_lowering=False, debug=False,
                   num_devices=N_CORES)
    x_d = nc.dram_tensor("x_in", [BPC, IN_DIM], F16, kind="ExternalInput").ap()
    rhs_d = nc.dram_tensor("rhs_in", [128, NG * 512], F16,
                           kind="ExternalInput").ap()
    out_d = nc.dram_tensor("out", [BPC, NG * 512], F16,
                           kind="ExternalOutput").ap()

    with tile.TileContext(nc) as tc, ExitStack() as ctx:
        pool = ctx.enter_context(tc.tile_pool(name="main", bufs=1))
        psT = ctx.enter_context(tc.tile_pool(name="psT", bufs=1, space="PSUM"))
        psO = ctx.enter_context(tc.tile_pool(name="psO", bufs=4, space="PSUM"))

        ident = pool.tile([128, 128], F16)
        make_identity(nc, ident)

        x_sb = pool.tile([BPC, IN_DIM], F16)
        rhs_sb = pool.tile([128, NG * 512], F16)
        nc.sync.dma_start(out=x_sb[:], in_=x_d)
        nc.sync.dma_start(out=rhs_sb[:, :4 * 512], in_=rhs_d[:, :4 * 512])
        nc.sync.dma_start(out=rhs_sb[:, 4 * 512:], in_=rhs_d[:, 4 * 512:])

        A2 = pool.tile([BPC, IN_DIM, 12], F16)
        sig = pool.tile([BPC, IN_DIM, 12], F16)
        r2n = pool.tile([BPC, IN_DIM, 12], F16)
        r1n = pool.tile([BPC, IN_DIM, 12], F16)
        sq2 = pool.tile([BPC, IN_DIM, 12], F16)
        sq1 = pool.tile([BPC, IN_DIM, 12], F16)
        cu2 = pool.tile([BPC, IN_DIM, 12], F16)
        cu1 = pool.tile([BPC, IN_DIM, 12], F16)
        c4 = pool.tile([BPC, IN_DIM, 12], F16)
        B3h = pool.tile([BPC, IN_DIM, K16], F16)

        # pad knots 12..15 feed the transpose -> real coef columns: keep 0
        nc.gpsimd.memset(B3h[:, :, 12:16], 0.0)

        V = nc.vector
        xv = x_sb[:].rearrange("p (i k) -> p i k", k=1)
        # A2[k] = x - g3[2+k]
        V.tensor_scalar_sub(A2[:, :, 0:1], xv, g3_2)
        V.tensor_scalar_sub(A2[:, :, 1:2], xv, g3_2 + h)
        V.tensor_scalar_sub(A2[:, :, 2:4], A2[:, :, 0:2], 2.0 * h)
        V.tensor_scalar_sub(A2[:, :, 4:8], A2[:, :, 0:4], 4.0 * h)
        V.tensor_scalar_sub(A2[:, :, 8:12], A2[:, :, 0:4], 8.0 * h)

        # sig = |A2| (unscaled; h^3 folds into the host coef scale)
        # r2n = min(|A2|-2h, 0) = -relu(2h-|A2|);  r1n likewise with h
        V.tensor_scalar_mul(c4[:], A2[:], -1.0)      # c4 reused as scratch
        V.tensor_max(sig[:], A2[:], c4[:])
        V.tensor_scalar(r2n[:], sig[:], 2.0 * h, 0.0, AL.subtract, AL.min)
        V.tensor_scalar(r1n[:], sig[:], 1.0 * h, 0.0, AL.subtract, AL.min)

        HW = IN_DIM // 2
        for H in range(2):
            s = slice(H * HW, (H + 1) * HW)
            V.tensor_mul(sq2[:, s, :], r2n[:, s, :], r2n[:, s, :])
            V.tensor_mul(sq1[:, s, :], r1n[:, s, :], r1n[:, s, :])
            V.tensor_mul(cu2[:, s, :], sq2[:, s, :], r2n[:, s, :])
            V.tensor_mul(cu1[:, s, :], sq1[:, s, :], r1n[:, s, :])
            V.tensor_scalar_mul(c4[:, s, :], cu1[:, s, :], 4.0)
            V.tensor_sub(B3h[:, s, 0:12], c4[:, s, :], cu2[:, s, :])

        basesT = pool.tile([128, NG * 128], F16)
        og = pool.tile([BPC, NG * 512], F16)
        ps0 = psT.tile([128, 4 * 128], F16)
        ps1 = psT.tile([128, 4 * 128], F16)

        B3f = B3h[:].rearrange("p i k -> p (i k)")
        for q in range(4):
            nc.tensor.transpose(out=ps0[:, q * 128:(q + 1) * 128],
                                in_=B3f[:, q * 128:(q + 1) * 128],
                                identity=ident[:])
        nc.scalar.copy(basesT[:, :4 * 128], ps0[:])

        cp = [nc.scalar.copy, nc.vector.tensor_copy]
        for g in range(4):
            ps_o = psO.tile([128, 512], F32)
            nc.tensor.matmul(out=ps_o[:],
                             lhsT=basesT[:, g * 128:(g + 1) * 128],
                             rhs=rhs_sb[:, g * 512:(g + 1) * 512],
                             start=True, stop=True)
            cp[g % 2](og[:, g * 512:(g + 1) * 512], ps_o[:])

        for q in range(4):
            nc.tensor.transpose(out=ps1[:, q * 128:(q + 1) * 128],
                                in_=B3f[:, (4 + q) * 128:(5 + q) * 128],
                                identity=ident[:])
        nc.vector.tensor_copy(basesT[:, 4 * 128:], ps1[:])

        for g in range(4, NG):
            ps_o = psO.tile([128, 512], F32)
            nc.tensor.matmul(out=ps_o[:],
                             lhsT=basesT[:, g * 128:(g + 1) * 128],
                             rhs=rhs_sb[:, g * 512:(g + 1) * 512],
                             start=True, stop=True)
            cp[g % 2](og[:, g * 512:(g + 1) * 512], ps_o[:])

        for j in range(4):
            nc.sync.dma_start(out=out_d[:, j * 1024:(j + 1) * 1024],
                              in_=og[:, j * 1024:(j + 1) * 1024])

    nc.compile()
    return nc


def _host_inputs(x, coef, grid):
    x = np.asarray(x, dtype=np.float32).astype(np.float16)
    coef = np.asarray(coef, dtype=np.float32)
    knots = np.asarray(grid, dtype=np.float32)[0, 0, :]          # (23,)
    h = float(knots[1] - knots[0])
    g3_2 = float(knots[10])

    cf = coef[:, :, 8:19].astype(np.float64) / (6.0 * h**3)      # (o, i, 11)
    # block-diagonal rhs per group: rows (i_l,m) x cols (i_l', o), K=128, N=512
    rhs = np.zeros((128, NG * 512), dtype=np.float16)
    for i_l in range(8):
        for g in range(NG):
            i = g * 8 + i_l
            rhs[i_l * 16:i_l * 16 + 11,
                g * 512 + i_l * 64:g * 512 + i_l * 64 + 64] = (
                    cf[:, i, :].T.astype(np.float16))
    return x, rhs, g3_2, h


def _execute(x, coef, grid, trace=False, **spmd_kwargs):
    xf, rhs, g3_2, h = _host_inputs(x, coef, grid)
    if "nc" not in _CACHE:
        _CACHE["nc"] = _build_nc(g3_2, h)
    nc = _CACHE["nc"]
    in_maps = [{"x_in": np.ascontiguousarray(xf[c * BPC:(c + 1) * BPC]),
                "rhs_in": rhs} for c in range(N_CORES)]
    res = run_bass_kernel_spmd(nc, in_maps, list(range(N_CORES)),
                               trace=trace, **spmd_kwargs)
    full = np.empty((B_TOT, OUT_DIM, IN_DIM), dtype=np.float32)
    for c in range(N_CORES):
        t = res.results[c]["out"].astype(np.float32)
        t = t.reshape(BPC, NG, 8, 64)                            # (b, g, i_l, o)
        full[c * BPC:(c + 1) * BPC] = (
            t.transpose(0, 3, 1, 2).reshape(BPC, OUT_DIM, IN_DIM))
    return full, res


def kernel(x, coef, grid):
    out, _ = _execute(x, coef, grid, trace=False)
    return out
